# revision 11
# baseline (speedup 1.0000x reference)
"""CLoRALinear Trainium2 kernel.

Computes y = x @ (W + (alpha/r) * A @ B.T).T + bias for
x:[4,2048,4096] f32, W:[4096,4096], bias:[4096], A:[4096,32], B:[4096,32].

Strategy: data-parallel over tokens across 8 NeuronCores (1024 tokens each).
Per core, in bf16 with fp32 PSUM accumulation:
  y_tile[128t, 512o] = sum_k x.T_k[:,m].T @ W.T_k[:,n]   (32 k-tiles)
                     + u_aug[:,m].T @ A_aug[:,n]          (LoRA + bias, K=33)
where u_aug rows 0:32 = (x @ B).T and row 32 = 1.0; A_aug rows 0:32 = A.T and
row 32 = bias.  alpha/r == 1.0, so no scale factor is applied.

x.T and W.T tiles are produced on-chip by PE transposes (fp32 inputs have no
DMA-transpose path; fp32->bf16 casts ride the SWDGE loads).  To keep the PE
HAM clock warm, W.T transposes for slice n+1 are interleaved after the m-tile
matmul groups of slice n instead of running as one long burst, and transpose
results are batched 4-per-PSUM-bank with a single copy out (alternating
DVE/ACT) so copies never gate the PE.
"""

import sys

sys.path.insert(0, "/opt/trn_rl_repo")

import numpy as np

import concourse.bass as bass
import concourse.tile as tile
from concourse import bacc, mybir
from concourse.bass_utils import run_bass_kernel_spmd
from concourse.masks import make_identity

F32 = mybir.dt.float32
BF16 = mybir.dt.bfloat16

N_CORES = 8
TOK = 1024          # tokens per core
DIN = 4096
DOUT = 4096
R = 32
KT = DIN // 128     # 32 k-tiles
MT = TOK // 128     # 8 m-tiles
NSL = 512           # out-features per n-slice
NT = DOUT // NSL    # 8 n-slices
CPS = NSL // 128    # 4 weight chunks per n-slice

_cached = None


def _build():
    nc = bacc.Bacc("TRN2", target_bir_lowering=False, debug=False)

    x_d = nc.dram_tensor("x", [TOK, DIN], F32, kind="ExternalInput").ap()
    w_d = nc.dram_tensor("weight", [DOUT, DIN], F32, kind="ExternalInput").ap()
    bias_d = nc.dram_tensor("bias", [DOUT], F32, kind="ExternalInput").ap()
    a_d = nc.dram_tensor("A", [DOUT, R], F32, kind="ExternalInput").ap()
    b_d = nc.dram_tensor("B", [DIN, R], F32, kind="ExternalInput").ap()
    y_d = nc.dram_tensor("out", [TOK, DOUT], F32, kind="ExternalOutput").ap()

    with tile.TileContext(nc) as tc:
        with (
            tc.tile_pool(name="const", bufs=1) as const_pool,
            tc.tile_pool(name="xchunk", bufs=2) as xchunk_pool,
            tc.tile_pool(name="wchunk", bufs=3) as wchunk_pool,
            tc.tile_pool(name="wT", bufs=2) as wT_pool,
            tc.tile_pool(name="yout", bufs=3) as y_pool,
            tc.tile_pool(name="tpsum", bufs=6, space="PSUM") as tpsum_pool,
            tc.tile_pool(name="ypsum", bufs=2, space="PSUM") as ypsum_pool,
        ):
            ident = const_pool.tile([128, 128], BF16)
            make_identity(nc, ident[:])
            ident_f32 = const_pool.tile([128, 128], F32)
            make_identity(nc, ident_f32[:])

            copy_idx = [0]

            def tcopy(dst, src):
                if copy_idx[0] % 2 == 0:
                    nc.vector.tensor_copy(dst, src)
                else:
                    nc.scalar.copy(dst, src)
                copy_idx[0] += 1

            def transpose_batch(dst3, src_chunk, kb):
                """Transpose 4 [128,128] blocks kb*4..kb*4+3 of src_chunk into
                one PSUM bank, then one copy out to dst3 ([128,4,128])."""
                f32 = src_chunk.dtype == F32
                pt = tpsum_pool.tile([128, 4, 128], F32 if f32 else BF16, tag="t")
                idn = ident_f32 if f32 else ident
                for j in range(4):
                    k = kb * 4 + j
                    nc.tensor.transpose(
                        pt[:, j, :], src_chunk[:, k * 128:(k + 1) * 128], idn[:]
                    )
                tcopy(dst3, pt[:])

            # ---- x phase: build x.T resident + u_aug; also W.T slice 0 ----
            x_t = const_pool.tile([128, KT, TOK], BF16)
            u_aug = const_pool.tile([R + 1, TOK], BF16)
            nc.gpsimd.memset(u_aug[R:R + 1, :], 1.0)

            # constants: B (natural, one DMA), A_aug = [A.T ; bias]
            b_all = const_pool.tile([128, KT, R], BF16)
            nc.gpsimd.dma_start(
                b_all[:], b_d.rearrange("(k p) r -> p k r", p=128)
            )
            a_nat = const_pool.tile([128, DOUT // 128, R], BF16)
            nc.gpsimd.dma_start(
                a_nat[:], a_d.rearrange("(o p) r -> p o r", p=128)
            )
            a_aug = const_pool.tile([R + 1, DOUT], BF16)
            nc.gpsimd.dma_start(a_aug[R:R + 1, :], bias_d[None, :])
            for o in range(DOUT // 128):
                pt = tpsum_pool.tile([R, 128], BF16, tag="t")
                nc.tensor.transpose(pt[:], a_nat[:, o, :], ident[:])
                nc.vector.tensor_copy(a_aug[0:R, o * 128:(o + 1) * 128], pt[:])

            w_t = [
                wT_pool.tile([128, KT, NSL], BF16, tag="wt", name="wt0"),
                wT_pool.tile([128, KT, NSL], BF16, tag="wt", name="wt1"),
            ]
            w_chunks = {}

            def load_w_chunk(n, c):
                ch = wchunk_pool.tile([128, DIN], BF16, tag="wchunk")
                nc.gpsimd.dma_start(
                    ch[:], w_d[n * NSL + c * 128:n * NSL + (c + 1) * 128, :]
                )
                w_chunks[(n, c)] = ch

            def wT_half(n, c, half):
                """Transpose half (16 blocks) of chunk c of slice n into w_t."""
                dst = w_t[n % 2]
                ch = w_chunks[(n, c)]
                for kb in range(half * 4, half * 4 + 4):
                    transpose_batch(
                        dst[:, kb * 4:(kb + 1) * 4, c * 128:(c + 1) * 128],
                        ch, kb,
                    )

            for m in range(MT):
                if m % 2 == 0:
                    x_chunk = xchunk_pool.tile(
                        [128, DIN], BF16, tag="xchunk", name="x_chunk"
                    )
                    nc.gpsimd.dma_start(
                        x_chunk[:, 0:DIN // 2],
                        x_d[m * 128:(m + 1) * 128, 0:DIN // 2],
                    )
                    nc.gpsimd.dma_start(
                        x_chunk[:, DIN // 2:],
                        x_d[m * 128:(m + 1) * 128, DIN // 2:],
                    )
                else:
                    xf = xchunk_pool.tile(
                        [128, DIN], F32, tag="xf32", name="xf", bufs=1
                    )
                    nc.sync.dma_start(xf[:], x_d[m * 128:(m + 1) * 128, :])
                    x_chunk = xchunk_pool.tile(
                        [128, DIN], BF16, tag="xchunk", name="x_chunk"
                    )
                    nc.vector.tensor_copy(x_chunk[:, 0:DIN // 2], xf[:, 0:DIN // 2])
                    nc.vector.tensor_copy(x_chunk[:, DIN // 2:], xf[:, DIN // 2:])
                if 1 <= m <= CPS:
                    load_w_chunk(0, m - 1)
                for kb in range(KT // 4):
                    transpose_batch(
                        x_t[:, kb * 4:(kb + 1) * 4, m * 128:(m + 1) * 128],
                        x_chunk, kb,
                    )
                if m >= 4:
                    # build W.T slice 0: chunk m-4, both halves
                    wT_half(0, m - 4, 0)
                    wT_half(0, m - 4, 1)

            # u = (x @ B).T over full token range, batched N=512 matmuls
            # (PSUM tiles borrowed from the ypsum pool slots)
            for mc in range(2):
                up = ypsum_pool.tile([R, NSL], F32, tag="y", name="up")
                for k in range(KT):
                    nc.tensor.matmul(
                        up[:],
                        b_all[:, k, :],
                        x_t[:, k, mc * NSL:(mc + 1) * NSL],
                        start=(k == 0),
                        stop=(k == KT - 1),
                    )
                tcopy(u_aug[0:R, mc * NSL:(mc + 1) * NSL], up[:])

            # ---- main loop over output-feature slices ----
            for n in range(NT):
                cur = w_t[n % 2]
                for m in range(MT):
                    if n + 1 < NT:
                        # chunk c of slice n+1 is consumed at m=2c and 2c+1;
                        # load it one m-iteration ahead (c=0 at m=0).
                        if m == 0:
                            load_w_chunk(n + 1, 0)
                        if m % 2 == 1 and (m + 1) // 2 < CPS:
                            load_w_chunk(n + 1, (m + 1) // 2)
                    yp = ypsum_pool.tile([128, NSL], F32, tag="y")
                    for k in range(KT):
                        nc.tensor.matmul(
                            yp[:],
                            x_t[:, k, m * 128:(m + 1) * 128],
                            cur[:, k, :],
                            start=(k == 0),
                            stop=False,
                        )
                    nc.tensor.matmul(
                        yp[:],
                        u_aug[:, m * 128:(m + 1) * 128],
                        a_aug[:, n * NSL:(n + 1) * NSL],
                        start=False,
                        stop=True,
                        skip_group_check=True,
                    )
                    y_sb = y_pool.tile([128, NSL], F32, tag="ysb")
                    nc.scalar.copy(y_sb[:], yp[:])
                    nc.sync.dma_start(
                        y_d[m * 128:(m + 1) * 128, n * NSL:(n + 1) * NSL],
                        y_sb[:],
                    )
                    if n + 1 < NT:
                        # 16 transposes of slice n+1 after each m's matmuls
                        wT_half(n + 1, m // 2, m % 2)

    nc.compile()
    return nc


def _get_nc():
    global _cached
    if _cached is None:
        _cached = _build()
    return _cached


def kernel(x, weight, bias, A, B, _trace=False):
    x = np.ascontiguousarray(np.asarray(x, dtype=np.float32)).reshape(-1, DIN)
    weight = np.ascontiguousarray(np.asarray(weight, dtype=np.float32))
    bias = np.ascontiguousarray(np.asarray(bias, dtype=np.float32))
    A = np.ascontiguousarray(np.asarray(A, dtype=np.float32))
    B = np.ascontiguousarray(np.asarray(B, dtype=np.float32))

    nc = _get_nc()
    in_maps = [
        {
            "x": np.ascontiguousarray(x[c * TOK:(c + 1) * TOK]),
            "weight": weight,
            "bias": bias,
            "A": A,
            "B": B,
        }
        for c in range(N_CORES)
    ]
    res = run_bass_kernel_spmd(
        nc, in_maps, core_ids=list(range(N_CORES)), trace=_trace
    )
    kernel.last_result = res
    y = np.concatenate([res.results[c]["out"] for c in range(N_CORES)], axis=0)
    return y.reshape(4, 2048, DOUT)


kernel.last_result = None
